# revision 38
# baseline (speedup 1.0000x reference)
"""Multi-head self-attention (B=2, S=2048, D=1024, H=16) on 8 Trainium2 NeuronCores.

Sharding: batch x head-group. Core c = b*4 + g handles batch b and heads 4g..4g+3
(Megatron-style TP: Wq/Wk/Wv column-sharded, Wo row-sharded; partial outputs
summed on the host).

v4 design (bf16 compute, fp32 PSUM accumulation), T-layout (sequence on the
free dim everywhere):
  QT/KT = (w.T @ xt) [256, 2048]      d' on partitions
  V     = (xt.T @ wv) [2048, 256]     natural layout
  scoresT[k, q] = KT_h.T @ QT_h       per head, K=64 row pairs (2 heads
                                      concurrent in rows 0-63 / 64-127)
  expT = exp(scoresT / 8)             bf16, scalar engine (no max subtraction:
                                      |scores| <~ 2)
  ctxT+den fused: va holds [V0 | ones | V1] (192 cols per head pair, ones
               shared); head0's stationary = cols 0:128 -> out rows 0-63 =
               ctx0, rows 64-127 = den0 (pre-broadcast); head1's = cols
               64:192 -> rows 0-63 = den1, rows 64-127 = ctx1. Matmul cost
               is free-size only, so the denominator rows are FREE (vs a
               separate ones-matmul stream costing 25% of PE time).
  outT_partial = wo.T @ ctxT          bf16 out, host sums partials

Scheduling: the PE (~173us busy) and the scalar-engine exp stream (~134us)
are co-paced; every stall costs double because the tensor engine drops out
of max p-state (~2x slower for ~3us) after any idle gap. Hence:
  - ~36 scratch matmuls warm the PE through the ~10us DMA/init phase; 14
    more cover the final normalize so the tail runs at full speed.
  - input DMA on one queue in strict priority order (wk, xs n0 slices, wq,
    wv, xs n1-n3, wo) so the first score chain is gated on 2MB, not 6.5MB.
  - block order (0,0),(0,1),(0,2),(0,3),(1,1),(1,2),(1,3),(1,0): po-chain
    fill work exists for every late block and only po(.,0) remains after
    the last block.
  - projection chains run as single-matmul pieces 1-2 per kc (ps_pj bank),
    po chains as 2-matmul lumps every other kc (ps_o bank), so the PE
    stream never starves between exp-gated scores.
  - ctx(kc) is emitted at iteration kc+3 (CD): the previous block's 5-op
    DVE normalize drains while the PE chews the next block's fills/scores.
  - tail po(.,0) chains spread over 8 distinct PSUM banks (4 score banks +
    the 2 freed ctx banks + pj + po) so no chain waits on a sibling evac.
"""
import sys

sys.path.insert(0, "/opt/trn_rl_repo")

import numpy as np
import ml_dtypes

import concourse.bass as bass
import concourse.tile as tile
from concourse import bacc, mybir
from concourse.bass_utils import run_bass_kernel_spmd

F32 = mybir.dt.float32
BF16 = mybir.dt.bfloat16

S = 2048          # sequence length per batch
D = 1024          # embedding dim
HG = 4            # heads per core
HD = 64           # head dim
GC = HG * HD      # group cols = 256
P = 128
NQ = 4            # q chunks of 512
QW = 512          # q chunk width
NKC = 16          # key-position chunks of 128
KO = 8            # contraction chunks of 128 over D

_NC_CACHE = {}


def _build():
    if "nc" in _NC_CACHE:
        return _NC_CACHE["nc"]
    nc = bacc.Bacc(trn_type="TRN2", target_bir_lowering=False, debug=False)
    # weights arrive host-swizzled to the SBUF layout so each DMA is one
    # contiguous 4KB-per-partition stream (512B packets otherwise gate the
    # first matmul on a slow scattered transfer)
    xt_d = nc.dram_tensor("xt", [D, S], BF16, kind="ExternalInput")
    wq_d = nc.dram_tensor("wq", [P, KO * GC], BF16, kind="ExternalInput")
    wk_d = nc.dram_tensor("wk", [P, KO * GC], BF16, kind="ExternalInput")
    wv_d = nc.dram_tensor("wv", [P, KO * GC], BF16, kind="ExternalInput")
    wo_d = nc.dram_tensor("wo", [P, 2 * D], BF16, kind="ExternalInput")
    out_d = nc.dram_tensor("out_t", [D, S], BF16, kind="ExternalOutput")
    with tile.TileContext(nc) as tc:
        _emit(nc, tc, xt_d, wq_d, wk_d, wv_d, wo_d, out_d)
    nc.compile()
    _NC_CACHE["nc"] = nc
    return nc


def _emit(nc, tc, xt_d, wq_d, wk_d, wv_d, wo_d, out_d):
    with tc.tile_pool(name="big", bufs=1) as big, \
         tc.tile_pool(name="expp", bufs=6) as expp, \
         tc.tile_pool(name="norm", bufs=2) as norm, \
         tc.tile_pool(name="evac", bufs=2) as evac, \
         tc.tile_pool(name="outp", bufs=4) as outp, \
         tc.tile_pool(name="ps_sc", bufs=2, space="PSUM") as ps_sc, \
         tc.tile_pool(name="ps_ctx", bufs=1, space="PSUM") as ps_ctx, \
         tc.tile_pool(name="ps_pj", bufs=1, space="PSUM") as ps_pj, \
         tc.tile_pool(name="ps_o", bufs=1, space="PSUM") as ps_o:
        # ---- persistent SBUF tensors (~75KB/partition, bf16) ----
        xs = big.tile([P, KO, S], BF16)         # x.T, [d_in(128) x ko x s]
        wqs = big.tile([P, KO, GC], BF16)
        wks = big.tile([P, KO, GC], BF16)
        wvs = big.tile([P, KO, GC], BF16)
        wos = big.tile([P, 2, D], BF16)         # [d'(128) x chunk x e]
        qt = big.tile([P, 2, S], BF16)          # head h at parts (h%2)*64, chunk h//2
        kt = big.tile([P, 2, S], BF16)
        # V per head pair: [V_e0 (64) | ones (64) | V_e1 (64)]; head e's
        # ctx stationary is va[:, kc, hp, e*64 : e*64+128]
        va = big.tile([P, NKC, 2, 3 * HD], BF16)
        ct = big.tile([P, 2, S], BF16)          # ctxT, same head layout as qt

        # DMA ordering: HBM BW (~360 B/ns) makes the full input ~18us. One
        # queue, strict priority order (two queues would share the engines
        # and halve the critical stream): wk -> xs n0 slices (lead-in KT) ->
        # wq (QT) -> wv (v_chains) -> xs n1/n2/n3 column-major (v_chain(kc)
        # needs all ko of column chunk kc) -> wo (first used ~100us in).
        xt_r = xt_d.rearrange("(ko p) s -> p ko s", p=P)
        nc.sync.dma_start(wks[:].rearrange("p ko m -> p (ko m)"), wk_d[:])
        for ko in range(KO):
            nc.sync.dma_start(xs[:, ko, 0:QW], xt_r[:, ko, 0:QW])
        nc.sync.dma_start(wqs[:].rearrange("p ko m -> p (ko m)"), wq_d[:])
        nc.sync.dma_start(wvs[:].rearrange("p ko m -> p (ko m)"), wv_d[:])
        for nn in range(1, NQ):
            for ko in range(KO):
                nc.sync.dma_start(xs[:, ko, nn * QW:(nn + 1) * QW],
                                  xt_r[:, ko, nn * QW:(nn + 1) * QW])
        nc.sync.dma_start(wos[:].rearrange("p c e -> p (c e)"), wo_d[:])

        # shared ones block of each [V0 | ones | V1] pair group (bf16 1.0)
        nc.vector.memset(va[:, :, :, HD:2 * HD].bitcast(mybir.dt.uint16), 0x3F80)

        def proj_chain(w_sb, m, n, dst):
            """dst[:, m, n*QW:] = sum_ko w_sb[:,ko,m*128:+128].T @ xs[:,ko,nq]"""
            pp = ps_pj.tile([P, QW], F32, tag="pj", name=f"pj_{m}_{n}")
            for ko in range(KO):
                nc.tensor.matmul(pp[:], w_sb[:, ko, m * P:(m + 1) * P],
                                 xs[:, ko, n * QW:(n + 1) * QW],
                                 start=(ko == 0), stop=(ko == KO - 1))
            nc.vector.tensor_copy(dst[:, m, n * QW:(n + 1) * QW], pp[:])

        def proj_pieces(w_sb, m, n, dst):
            """proj_chain split into 8 single-matmul thunks (spread 1/kc so
            the PE never idles long enough to drop out of max p-state)."""
            cell = {}

            def piece(ko):
                def t():
                    if ko == 0:
                        cell["pp"] = ps_pj.tile([P, QW], F32, tag="pj",
                                                name=f"pjp_{m}_{n}")
                    nc.tensor.matmul(cell["pp"][:],
                                     w_sb[:, ko, m * P:(m + 1) * P],
                                     xs[:, ko, n * QW:(n + 1) * QW],
                                     start=(ko == 0), stop=(ko == KO - 1),
                                     skip_group_check=True)
                    if ko == KO - 1:
                        nc.vector.tensor_copy(
                            dst[:, m, n * QW:(n + 1) * QW], cell["pp"][:])
                return t
            return [piece(ko) for ko in range(KO)]

        def v_chain(kc):
            """va[:, kc, hp, {0:64, 128:192}] = V columns (natural layout)"""
            pv = ps_o.tile([P, QW], F32, tag="po", name=f"pv_{kc}")
            for ko in range(KO):
                nc.tensor.matmul(pv[:, 0:GC], xs[:, ko, kc * P:(kc + 1) * P],
                                 wvs[:, ko, :],
                                 start=(ko == 0), stop=(ko == KO - 1))
            pv_r = pv[:, 0:GC].rearrange("p (a e d) -> p a e d", a=2, e=2)
            nc.vector.tensor_copy(va[:, kc, :, 0:HD], pv_r[:, :, 0, :])
            nc.vector.tensor_copy(va[:, kc, :, 2 * HD:3 * HD], pv_r[:, :, 1, :])

        def po_evac(pp, mo, n):
            # scalar-queue DMA issue serializes ~1.3us per transfer, so keep
            # all evacs on vector + the sync DMA queue
            ot = outp.tile([P, QW], BF16, tag="ot")
            nc.vector.tensor_copy(ot[:], pp)
            nc.sync.dma_start(
                out_d[mo * P:(mo + 1) * P, n * QW:(n + 1) * QW], ot[:])

        def po_chain(mo, n, pool=None):
            """out_t[mo*128:+128, nq] = sum_c wos[:,c,mo*128:+128].T @ ct[:,c,nq]"""
            pool = pool if pool is not None else ps_o
            tg = "pj" if pool is ps_pj else "po"
            pp = pool.tile([P, QW], F32, tag=tg, name=f"po_{mo}_{n}")
            for c in range(2):
                nc.tensor.matmul(pp[:], wos[:, c, mo * P:(mo + 1) * P],
                                 ct[:, c, n * QW:(n + 1) * QW],
                                 start=(c == 0), stop=(c == 1))
            po_evac(pp[:], mo, n)

        # ---- PE warm-up: the tensor engine needs ~3us of continuous work
        # to reach max p-state, and the input DMA takes ~10us before the
        # lead-in can start. Burn scratch matmuls (no data deps, reading
        # whatever is in ct/qt) so the PE is hot when real work arrives.
        wu = ps_sc.tile([P, 2, QW], F32, tag="psc", name="warmup")
        for i in range(36):
            nc.tensor.matmul(wu[:, i % 2, :], ct[:, 0, 0:P], qt[:, 0, 0:QW],
                             start=True, stop=True, skip_group_check=True)

        # ---- lead-in: KT chain first (needs only wk + the n0 xs slices,
        # which the DMA order delivers first), then QT once wq has landed.
        ktp = ps_sc.tile([P, 2, QW], F32, tag="psc", name="lead_k")
        for ko in range(KO):
            nc.tensor.matmul(ktp[:, 0, :], wks[:, ko, 0:P], xs[:, ko, 0:QW],
                             start=(ko == 0), stop=(ko == KO - 1))
        for ko in range(KO):
            nc.tensor.matmul(ktp[:, 1, :], wqs[:, ko, 0:P], xs[:, ko, 0:QW],
                             start=(ko == 0), stop=(ko == KO - 1))
        nc.vector.tensor_copy(kt[:, 0, 0:QW], ktp[:, 0, :])
        nc.vector.tensor_copy(qt[:, 0, 0:QW], ktp[:, 1, :])

        # ---- fill schedule: per block, kc -> [thunks]. Projections run as
        # single-matmul pieces (1/kc) and po chains as 2-matmul lumps every
        # other kc, so the PE stream has no long idle gaps (idle > ~0.5us
        # drops the tensor engine out of max p-state for ~3us). Block order
        # ends on (1,0) so po(.,3) fills the last block and po(.,0) is the
        # only tail work.
        blocks = [(0, 0), (0, 1), (0, 2), (0, 3),
                  (1, 1), (1, 2), (1, 3), (1, 0)]
        fills = {b: {} for b in blocks}

        def add(block, kc0, thunks, stride=1):
            for i, t in enumerate(thunks):
                fills[block].setdefault(kc0 + stride * i, []).append(t)

        # (0,0): data-critical K/Q m0 chains stay lumped (v_chain supplies
        # per-kc PE work there already; the lumps must finish fast).
        add((0, 0), 0, [lambda: proj_chain(wks, 0, 1, kt)])
        add((0, 0), 4, [lambda: proj_chain(wks, 0, 2, kt)])
        add((0, 0), 8, [lambda: proj_chain(wks, 0, 3, kt)])
        add((0, 0), 10, [lambda: proj_chain(wqs, 0, 1, qt)])
        add((0, 1), 0, proj_pieces(wqs, 0, 2, qt))
        add((0, 1), 8, proj_pieces(wks, 1, 0, kt))
        add((0, 2), 0, proj_pieces(wqs, 0, 3, qt))
        add((0, 2), 8, proj_pieces(wks, 1, 1, kt))
        add((0, 3), 0, proj_pieces(wqs, 1, 1, qt))
        # two chains share the single ps_pj bank: run them sequentially at
        # 2 pieces/kc (n2 over kc 8-11, n3 over kc 12-15)
        p_n2 = proj_pieces(wks, 1, 2, kt)
        p_n3 = proj_pieces(wks, 1, 3, kt)
        add((0, 3), 8, p_n2[0::2])
        add((0, 3), 8, p_n2[1::2])
        add((0, 3), 12, p_n3[0::2])
        add((0, 3), 12, p_n3[1::2])
        add((1, 1), 0, proj_pieces(wqs, 1, 2, qt))
        add((1, 1), 8, proj_pieces(wqs, 1, 0, qt))
        add((1, 2), 0, proj_pieces(wqs, 1, 3, qt))
        add((1, 2), 0, [lambda mo=mo: po_chain(mo, 1) for mo in range(8)],
            stride=2)
        add((1, 3), 0, [lambda mo=mo: po_chain(mo, 2) for mo in range(8)],
            stride=2)
        add((1, 0), 0, [lambda mo=mo: po_chain(mo, 3) for mo in range(8)],
            stride=2)

        def emit_scores(hp, n, kc, sps):
            sp = ps_sc.tile([P, 2, QW], F32, tag="psc", name=f"sp_{hp}_{n}_{kc}")
            for e in range(2):   # head 2hp+e in rows e*64..e*64+63
                lo = e * HD
                nc.tensor.matmul(
                    sp[:, e, :],
                    kt[lo:lo + HD, hp, kc * P:(kc + 1) * P],
                    qt[lo:lo + HD, hp, n * QW:(n + 1) * QW],
                    start=True, stop=True)
            sps[kc] = sp

        CD = 3   # ctx delay: ctx(kc) is emitted in iteration kc+CD so the
        #          previous block's normalize never stalls the PE stream
        sps_carry = {}
        for bi, (hp, n) in enumerate(blocks):
            sched = fills[(hp, n)]
            # bank e holds head e: [ctx0|den0] rows (0-63|64-127) for e=0,
            # [den1|ctx1] for e=1 (ctx rows match the ct head layout)
            cdp = ps_ctx.tile([P, 2, QW], F32, tag="pc", name=f"pc_{hp}_{n}")
            sps = sps_carry
            sps_carry = {}
            if bi == 0:
                emit_scores(hp, n, 0, sps)
                emit_scores(hp, n, 1, sps)

            def ctx_pair(kc):
                ex = exs.pop(kc)
                for e in range(2):   # fused ctx+den: M=128 costs the same
                    nc.tensor.matmul(
                        cdp[:, e, :],
                        va[:, kc, hp, e * HD:e * HD + 2 * HD],
                        ex[:, e, :],
                        start=(kc == 0), stop=(kc == NKC - 1),
                        skip_group_check=True)

            exs = {}
            for kc in range(NKC):
                sp = sps.pop(kc)
                ex = expp.tile([P, 2, QW], BF16, tag="pex")
                nc.scalar.activation(
                    ex[:].rearrange("p a b -> p (a b)"),
                    sp[:].rearrange("p a b -> p (a b)"),
                    mybir.ActivationFunctionType.Exp,
                    scale=0.125)
                exs[kc] = ex
                if bi == 0:
                    # feed the starved exp stream first, then v/fills
                    if kc + 2 < NKC:
                        emit_scores(hp, n, kc + 2, sps)
                    elif bi + 1 < len(blocks):
                        emit_scores(*blocks[bi + 1], kc + 2 - NKC, sps_carry)
                    v_chain(kc)
                    for t in sched.get(kc, []):
                        t()
                else:
                    if kc == 0 and bi >= 6:
                        # (1,3)/(1,0) enter with only norm-gated po fills;
                        # scratch matmuls (idle pj bank) bridge the previous
                        # block's normalize so the PE keeps max p-state
                        pjw = ps_pj.tile([P, QW], F32, tag="pj",
                                         name=f"bwu_{bi}")
                        for _ in range(8 if bi == 7 else 4):
                            nc.tensor.matmul(pjw[:], qt[:, 0, 0:P],
                                             qt[:, 0, 0:QW],
                                             start=True, stop=True,
                                             skip_group_check=True)
                    for t in sched.get(kc, []):
                        t()
                    if kc + 2 < NKC:
                        emit_scores(hp, n, kc + 2, sps)
                    elif bi + 1 < len(blocks):
                        emit_scores(*blocks[bi + 1], kc + 2 - NKC, sps_carry)
                if kc >= CD:
                    ctx_pair(kc - CD)
            for kc in range(NKC - CD, NKC):
                ctx_pair(kc)
            # normalize: dens sit on the opposite partition half from
            # their ctx rows; tensor_copy does the partition shift (the
            # custom-DVE reciprocal reads the wrong partitions if shifted)
            dsb = norm.tile([P, QW], F32, tag="nd")
            nc.vector.tensor_copy(dsb[0:HD, :], cdp[HD:P, 0, :])
            nc.vector.tensor_copy(dsb[HD:P, :], cdp[0:HD, 1, :])
            rr = norm.tile([P, QW], F32, tag="nr")
            nc.vector.reciprocal_approx_fast(rr[:], dsb[:])
            nc.vector.tensor_tensor(
                ct[0:HD, hp, n * QW:(n + 1) * QW],
                cdp[0:HD, 0, :], rr[0:HD, :], mybir.AluOpType.mult)
            nc.vector.tensor_tensor(
                ct[HD:P, hp, n * QW:(n + 1) * QW],
                cdp[HD:P, 1, :], rr[HD:P, :], mybir.AluOpType.mult)
        # ---- tail: output projection of q-block 0, widened across the
        # now-idle scores PSUM banks so the chains pipeline. Scratch matmuls
        # keep the PE at max p-state while the last normalize drains.
        wu2 = ps_sc.tile([P, 2, QW], F32, tag="psc", name="warm_tail")
        for i in range(14):
            nc.tensor.matmul(wu2[:, i % 2, :], qt[:, 0, 0:P], kt[:, 0, 0:QW],
                             start=True, stop=True, skip_group_check=True)
        tp = ps_sc.tile([P, 2, QW], F32, tag="psc", name="tail_a")
        tp2 = ps_sc.tile([P, 2, QW], F32, tag="psc", name="tail_b")
        tcd = ps_ctx.tile([P, 2, QW], F32, tag="pc", name="tail_cd")
        banks = [tp[:, 0, :], tp[:, 1, :], tp2[:, 0, :], tp2[:, 1, :],
                 tcd[:, 0, :], tcd[:, 1, :]]
        # pj/po-bank chains first: their pool-recycle semaphore latency hides
        # under the final normalize instead of trailing the kernel
        for mo in range(KO):
            if mo < 2:
                po_chain(mo, 0, pool=(ps_pj if mo == 0 else ps_o))
            else:
                pp = banks[mo - 2]
                for c in range(2):
                    nc.tensor.matmul(pp, wos[:, c, mo * P:(mo + 1) * P],
                                     ct[:, c, 0:QW],
                                     start=(c == 0), stop=(c == 1))
                po_evac(pp, mo, 0)


def _in_maps(x, wq_f, wk_f, wv_f, wo_f):
    bf = ml_dtypes.bfloat16

    def swz(w):  # [1024, 256] -> [128, 8*256] SBUF layout (p, ko, m)
        return np.ascontiguousarray(
            w.reshape(KO, P, GC).transpose(1, 0, 2).reshape(P, KO * GC)).astype(bf)

    maps = []
    for core in range(8):
        b, g = core // 4, core % 4
        cols = slice(g * GC, (g + 1) * GC)
        wo_c = wo_f[cols, :]          # [256, 1024] -> [128, 2*1024] (p, c, e)
        maps.append({
            "xt": np.ascontiguousarray(x[b].T).astype(bf),
            "wq": swz(wq_f[:, cols]),
            "wk": swz(wk_f[:, cols]),
            "wv": swz(wv_f[:, cols]),
            "wo": np.ascontiguousarray(
                wo_c.reshape(2, P, D).transpose(1, 0, 2).reshape(P, 2 * D)).astype(bf),
        })
    return maps


def _prep(x, Wq, Wk, Wv, Wo, q_scale, k_scale, v_scale, o_scale):
    x = np.asarray(x, dtype=np.float32)
    wq_f = (np.asarray(Wq).T * np.asarray(q_scale).reshape(1, -1)).astype(np.float32)
    wk_f = (np.asarray(Wk).T * np.asarray(k_scale).reshape(1, -1)).astype(np.float32)
    wv_f = (np.asarray(Wv).T * np.asarray(v_scale).reshape(1, -1)).astype(np.float32)
    wo_f = (np.asarray(Wo).T * np.asarray(o_scale).reshape(1, -1)).astype(np.float32)
    return x, wq_f, wk_f, wv_f, wo_f


def _gather(res, B):
    out = np.zeros((B, S, D), dtype=np.float32)
    for core in range(8):
        out[core // 4] += res.results[core]["out_t"].astype(np.float32).T
    return out


def run_traced(x, Wq, Wk, Wv, Wo, q_scale, k_scale, v_scale, o_scale):
    """Like kernel() but with NTFF tracing; returns (out, exec_time_ns, trace_path)."""
    x, wq_f, wk_f, wv_f, wo_f = _prep(x, Wq, Wk, Wv, Wo,
                                      q_scale, k_scale, v_scale, o_scale)
    nc = _build()
    res = run_bass_kernel_spmd(nc, _in_maps(x, wq_f, wk_f, wv_f, wo_f),
                               core_ids=list(range(8)), trace=True)
    out = _gather(res, x.shape[0])
    trace_path = None
    if res.instructions_and_trace is not None:
        trace_path = res.instructions_and_trace[1]
    return out, res.exec_time_ns, trace_path


def kernel(x, Wq, Wk, Wv, Wo, q_scale, k_scale, v_scale, o_scale):
    B = x.shape[0]
    x, wq_f, wk_f, wv_f, wo_f = _prep(x, Wq, Wk, Wv, Wo,
                                      q_scale, k_scale, v_scale, o_scale)
    nc = _build()
    res = run_bass_kernel_spmd(nc, _in_maps(x, wq_f, wk_f, wv_f, wo_f),
                               core_ids=list(range(8)))
    return _gather(res, B)

